# revision 19
# baseline (speedup 1.0000x reference)
"""Trainium2 Bass/Tile kernel for an RNN-T Joiner:

    enc_p = encoder_out @ W_enc.T + b_enc          (N,200,512)
    dec_p = decoder_out @ W_dec.T + b_dec          (N,50,512)
    act   = tanh(enc_p[:,:,None,:] + dec_p[:,None,:,:])
    out   = act @ W_out.T + b_out                  (N,200,50,500)

Sharding: data-parallel over N=8 — core i computes batch element i end to
end; weights are replicated to every core. All device inputs are staged
host-side in the PE-friendly layout: contraction dim leading (pre-
transposed) and bf16 — the standard inference-deployment format. Biases
stay fp32; the output is fp32.

Per-core dataflow (everything on-chip after the initial loads):
  - a burst of dependency-free scratch matmuls fires at t~10us to warm the
    HAM clock governor (a cold PE runs at half rate until it sees ~3.4us
    of sustained matmul activity, and re-throttles after any >3.4us gap),
  - one consolidated DMA per input tensor (many small dma_starts serialize
    the prologue),
  - project:  enc_pT[j,t], dec_pT[j,u]  (PE bf16 -> fp32 PSUM, bias folded
    in via the ACT copy out of PSUM, stored bf16),
  - acts live in ONE whole-T buffer per j-block [128, 10000] bf16; they
    are produced in graded t-slices (16/16/32/64/64/8) — fine slices up
    front so the first vocab matmuls start ~4us after the projections,
    coarse later for efficiency. Broadcast-add: jb0/jb1 on DVE, jb2/jb3
    on GPSIMD (GPSIMD cannot address PSUM, so the DVE owns every PSUM
    drain; the two engines share an SBUF port and throttle each other
    ~2.5x when co-streaming, so the split keeps both just ahead of the
    PE). tanh on ACT chases each slice,
  - vocab matmul per 128-cell block: psum[cell,v] = sum_jb actT_blk.T @
    W_outT[jb] (bf16 -> fast weight load) — one long back-to-back MM
    stream over all 79 blocks,
  - +b_out fused into the PSUM->SBUF drain (DVE tensor_tensor with a
    pre-broadcast fp32 bias tile), output DMA in ~1.25MB batches.
"""

import numpy as np
from contextlib import ExitStack

N, T, U = 8, 200, 50
E = J = 512
V = 500
CELLS = T * U
P = 128
KB = J // P  # 4 contraction blocks

_NC_CACHE = {}


def _build_nc():
    import concourse.mybir as mybir
    import concourse.tile as tile
    from concourse import bacc

    f32 = mybir.dt.float32
    bf16 = mybir.dt.bfloat16
    ADD = mybir.AluOpType.add
    TANH = mybir.ActivationFunctionType.Tanh
    IDENT = mybir.ActivationFunctionType.Identity

    nc = bacc.Bacc("TRN2", target_bir_lowering=False, debug=False)

    encT_d = nc.dram_tensor("encT", [E, T], bf16, kind="ExternalInput").ap()
    decT_d = nc.dram_tensor("decT", [E, U], bf16, kind="ExternalInput").ap()
    wencT_d = nc.dram_tensor("W_encT", [E, J], bf16, kind="ExternalInput").ap()
    benc_d = nc.dram_tensor("b_enc", [J], f32, kind="ExternalInput").ap()
    wdecT_d = nc.dram_tensor("W_decT", [E, J], bf16, kind="ExternalInput").ap()
    bdec_d = nc.dram_tensor("b_dec", [J], f32, kind="ExternalInput").ap()
    woutT_d = nc.dram_tensor("W_outT", [J, V], bf16, kind="ExternalInput").ap()
    bout_d = nc.dram_tensor("b_out", [V], f32, kind="ExternalInput").ap()
    out_d = nc.dram_tensor("logits", [CELLS, V], f32, kind="ExternalOutput").ap()

    with tile.TileContext(nc) as tc, ExitStack() as ctx:
        const = ctx.enter_context(tc.tile_pool(name="const", bufs=1))
        pj_ps = ctx.enter_context(tc.tile_pool(name="pj_ps", bufs=2, space="PSUM"))
        mm_ps = ctx.enter_context(tc.tile_pool(name="mm_ps", bufs=3, space="PSUM"))
        act_pool = ctx.enter_context(tc.tile_pool(name="act", bufs=1))
        out_pool = ctx.enter_context(tc.tile_pool(name="outp", bufs=3))

        # ---- HAM pre-warm: dependency-free matmul burst ----
        # operands come from the framework's pre-memset const APs, so the
        # burst only waits on engine init (~5us), not on any DMA/memset
        wm_l = nc.const_aps.tensor(1.0, [P, P], bf16)
        wm_r = nc.const_aps.tensor(1.0, [P, V], bf16)
        wp = mm_ps.tile([P, 1024], f32, tag="mm", name="warm_ps")
        for i in range(18):
            nc.tensor.matmul(
                wp[:, :V], lhsT=wm_l, rhs=wm_r, start=True, stop=True
            )

        def load_rows(dram_ap, cols, name):
            # One DMA per tensor (21 small dma_starts serialized the
            # prologue to ~25us; consolidated loads finish much earlier).
            big = const.tile([P, KB * cols], bf16, name=f"{name}_all")
            nc.sync.dma_start(
                big[:].rearrange("p (kb c) -> p kb c", kb=KB),
                dram_ap.rearrange("(kb p) c -> p kb c", p=P),
            )
            return [big[:, kb * cols : (kb + 1) * cols] for kb in range(KB)]

        W_encT = load_rows(wencT_d, J, "wenc")  # 4 x [128(e), 512(j)]
        encT = load_rows(encT_d, T, "enc")      # 4 x [128(e), 200(t)]
        W_decT = load_rows(wdecT_d, J, "wdec")  # 4 x [128(e), 512(j)]
        decT = load_rows(decT_d, U, "dec")      # 4 x [128(e), 50(u)]
        W_outT = load_rows(woutT_d, V, "wout")  # 4 x [128(j), 500(v)]

        b_enc_sb = const.tile([P, KB], f32)
        nc.sync.dma_start(b_enc_sb[:], benc_d.rearrange("(kb p) -> p kb", p=P))
        b_dec_sb = const.tile([P, KB], f32)
        nc.sync.dma_start(b_dec_sb[:], bdec_d.rearrange("(kb p) -> p kb", p=P))

        # Projections -> enc_pT[jb]: [128(j), T] bf16, dec_pT[jb]: [128(j), U]
        def project(WT, srcT, b_sb, width, nm):
            outs = []
            for jb in range(KB):
                pp = pj_ps.tile([P, T], f32, tag="pj", name=f"{nm}_ps{jb}")
                for kb in range(KB):
                    nc.tensor.matmul(
                        pp[:, :width],
                        lhsT=WT[kb][:, jb * P : (jb + 1) * P],
                        rhs=srcT[kb][:, :width],
                        start=(kb == 0),
                        stop=(kb == KB - 1),
                    )
                o = const.tile([P, width], bf16, name=f"{nm}{jb}")
                nc.scalar.activation(o[:], pp[:, :width], IDENT, bias=b_sb[:, jb : jb + 1])
                outs.append(o)
            return outs

        enc_pT = project(W_encT, encT, b_enc_sb, T, "encp")
        dec_pT = project(W_decT, decT, b_dec_sb, U, "decp")

        # gap fillers: the PE reaches these (dependency-free) right after
        # the projections and runs them while the first act slices are
        # still being produced — no idle gap, no HAM re-throttle
        for i in range(10):
            nc.tensor.matmul(
                wp[:, :V], lhsT=wm_l, rhs=wm_r, start=True, stop=True
            )

        # ---- acts: one whole-T buffer per jb, produced in graded slices ----
        acts = [
            act_pool.tile([P, CELLS], bf16, tag=f"act{jb}", name=f"acts{jb}")
            for jb in range(KB)
        ]
        # 32-t steady slices: a 64-t slice is ~26us of DVE+ACT production
        # feeding ~21us of PE demand — the PE drains its PSUM lead and
        # stalls; 32-t halves the bubble
        SLICES = [
            (0, 16), (16, 16), (32, 32), (64, 32), (96, 32),
            (128, 32), (160, 32), (192, 8),
        ]

        def gen_slice(si):
            t0, L = SLICES[si]
            c0, C = t0 * U, L * U
            for jb in range(KB):
                s = acts[jb]
                # all adds on the DVE: GPSIMD runs them 2.5x slower AND
                # throttles concurrent DVE work via the shared SBUF port
                # (measured: net-negative contribution)
                nc.vector.tensor_tensor(
                    out=s[:, c0 : c0 + C].rearrange("p (l u) -> p l u", u=U),
                    in0=dec_pT[jb][:, None, :].broadcast_to([P, L, U]),
                    in1=enc_pT[jb][:, t0 : t0 + L][:, :, None].broadcast_to([P, L, U]),
                    op=ADD,
                )
                # tanh chases each slice (halves for the big slices)
                n_h = 2 if C >= 1600 else 1
                h = C // n_h
                for q in range(n_h):
                    nc.scalar.activation(
                        s[:, c0 + q * h : c0 + (q + 1) * h],
                        s[:, c0 + q * h : c0 + (q + 1) * h],
                        TANH,
                    )

        # slice si makes cells < CUM[si] available (slice boundaries in cells)
        CUM = [800, 1600, 3200, 4800, 6400, 8000, 9600, 10000]
        BATCH = 6  # output blocks per DMA (6*128 cells * 2000B = 1.54 MB)
        NFULL = CELLS // P  # 78 full blocks
        TAIL = CELLS % P    # 16

        gen_slice(0)
        gen_slice(1)
        next_slice = 2

        # b_out broadcast to all 128 partitions via a K=1 ones matmul.
        # bf16 copies of the ones column / bias row also serve as the
        # per-block bias matmuls (ACT-drained pairs fold b_out in PSUM via
        # K=1 accumulating matmuls instead of a DVE add).
        bout_row = const.tile([1, V], f32)
        nc.sync.dma_start(bout_row[:], bout_d[None, :])
        ones_col = const.tile([1, P], f32)
        nc.gpsimd.memset(ones_col[:], 1.0)
        ones_col_bf = const.tile([1, P], bf16)
        nc.vector.tensor_copy(ones_col_bf[:], ones_col[:])
        bout_row_bf = const.tile([1, V], bf16)
        nc.vector.tensor_copy(bout_row_bf[:], bout_row[:])
        bp = mm_ps.tile([P, 1024], f32, tag="mm")
        nc.tensor.matmul(bp[:, :V], lhsT=ones_col[:], rhs=bout_row[:], start=True, stop=True)
        bout_rep = const.tile([P, V], f32)
        nc.vector.tensor_copy(bout_rep[:], bp[:, :V])
        bout_rep2 = const.tile([P, 2 * V], f32)
        nc.vector.tensor_copy(bout_rep2[:, :V], bp[:, :V])
        nc.vector.tensor_copy(bout_rep2[:, V:], bp[:, :V])

        b0 = 0
        pair_idx = 0
        while b0 < NFULL:
            nb = min(BATCH, NFULL - b0)
            # emit act slices just-in-time: keep production one slice ahead
            # of the blocks being emitted
            while next_slice < len(SLICES) and (b0 + nb + 1) * P > CUM[next_slice - 1]:
                gen_slice(next_slice)
                next_slice += 1
            ob = out_pool.tile([P, BATCH * V], f32, tag="ob", name=f"ob{b0}")
            for pq in range(nb // 2):
                blk = b0 + 2 * pq
                # two blocks share a two-bank PSUM tile and drain with ONE
                # instruction — halves the drain count and the semaphore
                # traffic that was eating ~35us of DVE time. Pairs alternate
                # ACT (plain copy, b_out via K=1 matmuls) / DVE (bias in the
                # tensor_tensor) to balance the two engines.
                on_act = pair_idx % 2 == 0
                pair_idx += 1
                # [P, 1024]: PSUM banks hold 512 f32, so the two 500-wide
                # sub-blocks sit at offsets 0 and 512 (bank-aligned)
                ps = mm_ps.tile([P, 1024], f32, tag="mm", name=f"ps{blk}")
                for sub in range(2):
                    so = sub * 512
                    for jb in range(KB):
                        nc.tensor.matmul(
                            ps[:, so : so + V],
                            lhsT=acts[jb][:, (blk + sub) * P : (blk + sub + 1) * P],
                            rhs=W_outT[jb][:],
                            start=(jb == 0),
                            stop=(jb == KB - 1) and not on_act,
                        )
                    if on_act:
                        nc.tensor.matmul(
                            ps[:, so : so + V],
                            lhsT=ones_col_bf[:],
                            rhs=bout_row_bf[:],
                            start=False,
                            stop=True,
                        )
                q0 = 2 * pq * V
                ps_v = ps[:].rearrange("p (b v) -> p b v", v=512)[:, :, :V]
                ob_v = ob[:, q0 : q0 + 2 * V].rearrange("p (b v) -> p b v", v=V)
                if on_act:
                    nc.scalar.copy(ob_v, ps_v)
                else:
                    nc.vector.tensor_tensor(
                        out=ob_v,
                        in0=ps_v,
                        in1=bout_rep2[:].rearrange("p (b v) -> p b v", v=V),
                        op=ADD,
                    )
            c0 = b0 * P
            dst = out_d[c0 : c0 + nb * P, :].rearrange("(b p) v -> p b v", p=P)
            nc.sync.dma_start(dst, ob[:, : nb * V].rearrange("p (b v) -> p b v", v=V))
            b0 += nb
        while next_slice < len(SLICES):
            gen_slice(next_slice)
            next_slice += 1
        # tail block (16 cells)
        ps = mm_ps.tile([P, 1024], f32, tag="mm", name="ps_tail")
        for jb in range(KB):
            nc.tensor.matmul(
                ps[:TAIL, :V],
                lhsT=acts[jb][:, NFULL * P : NFULL * P + TAIL],
                rhs=W_outT[jb][:],
                start=(jb == 0),
                stop=(jb == KB - 1),
            )
        obt = out_pool.tile([P, BATCH * V], f32, tag="ob", name="ob_tail")
        nc.vector.tensor_tensor(
            out=obt[:TAIL, :V], in0=ps[:TAIL, :V], in1=bout_rep[:TAIL, :], op=ADD
        )
        nc.sync.dma_start(out_d[NFULL * P :, :], obt[:TAIL, :V])

    nc.compile()
    return nc


def get_nc():
    if "nc" not in _NC_CACHE:
        _NC_CACHE["nc"] = _build_nc()
    return _NC_CACHE["nc"]


def make_in_maps(inputs):
    import ml_dtypes

    bf16 = ml_dtypes.bfloat16

    def t_bf16(a):  # host-side: transpose + cast = accelerator staging layout
        return np.ascontiguousarray(np.asarray(a, dtype=np.float32).T).astype(bf16)

    enc = np.asarray(inputs["encoder_out"], dtype=np.float32)
    dec = np.asarray(inputs["decoder_out"], dtype=np.float32)
    shared = {
        "W_encT": t_bf16(inputs["W_enc"]),
        "W_decT": t_bf16(inputs["W_dec"]),
        "W_outT": t_bf16(inputs["W_out"]),
        "b_enc": np.ascontiguousarray(np.asarray(inputs["b_enc"], dtype=np.float32)),
        "b_dec": np.ascontiguousarray(np.asarray(inputs["b_dec"], dtype=np.float32)),
        "b_out": np.ascontiguousarray(np.asarray(inputs["b_out"], dtype=np.float32)),
    }
    return [
        {"encT": t_bf16(enc[i]), "decT": t_bf16(dec[i]), **shared} for i in range(N)
    ]


def kernel(**inputs):
    from concourse.bass_utils import run_bass_kernel_spmd

    nc = get_nc()
    in_maps = make_in_maps(inputs)
    res = run_bass_kernel_spmd(nc, in_maps, core_ids=list(range(N)))
    out = np.stack([r["logits"] for r in res.results], axis=0)
    return out.reshape(N, T, U, V)
